# revision 30
# baseline (speedup 1.0000x reference)
"""Trainium2 Bass kernel for DualTimeConstantHighPassMixAdaptation.

Math (reference):
    xr = relu(x)
    Mf[t] = (1-mu_f)*Mf[t-1] + mu_f*xr[t],  Mf[0] = xr[0]   (same for Ms)
    M  = ma*Mf + (1-ma)*Ms,   ma = sigmoid(mix_weight_adapt)
    out = xr/(eps+M) + mh*(xr - M),         mh = sigmoid(mix_weight_hp)

Kernel formulation:
    Pf = mh*ma*Mf, Ps = mh*(1-ma)*Ms, S = Pf + Ps + mh*eps = mh*(M+eps)
    q  = 1/S ;  t1 = (q+1)*mh*xr ;  out = t1 - S   (drops +mh*eps ~ 5.7e-7)

Each EMA chunk is ONE custom DVE op (bubble-free, ~1 elem/cycle/lane),
using the exponential-ramp diagonalization:
    Pf[t] = af^t * (af*carry + sum_k (Q*af^-k) * relu(x[k]))
body = scan(ADD, relu(Src0)*Src1, init=C0*C1) * geo(C1), where Src1 is a
precomputed decay ramp (constants folded in) and geo is a per-element
geometric ramp from the subdim-step scan (pages of 1).

Sharding: core b <- batch b (64 channel lanes). In-core the 64 lanes are
split into two time halves -> 128 partitions x 32000 samples. Half-1 rows
get their scan initial state from a prepass that re-scans the last
W samples of half-0 (EMA decay bounds the truncation error ~a^W).

Input loads / output stores are pairs of 2D HWDGE DMAs (a 3D access
pattern lands on only 2 of 16 SDMA engines; 2D spreads across all 16).
"""

import sys

for _p in ("/opt/trn_rl_repo", "/root/.axon_site/_ro/trn_rl_repo"):
    if _p not in sys.path:
        sys.path.insert(0, _p)

from contextlib import ExitStack

import numpy as np

import concourse.bacc as bacc
import concourse.tile as tile
from concourse import mybir
from concourse.bass_utils import run_bass_kernel_spmd

from concourse import dve_ops
from concourse.dve_spec import (AluOp as _DAlu, Bin as _DBin, Scan as _DScan,
                                Spec as _DSpec, Src0 as _Src0, Src1 as _Src1,
                                C0 as _C0, C1 as _C1, C2 as _C2, Zero as _DZero,
                                One as _DOne, relu as _drelu, lower as _dlower,
                                _has_src1)
from concourse.dve_uop import DveOpSpec as _DveOpSpec

_dt = mybir.dt.float32
_A = mybir.AluOpType
_Act = mybir.ActivationFunctionType

# Problem geometry (hardcoded per spec).
B, C, T = 8, 64, 64000
HALF = T // 2          # 32000
FT = 2000              # chunk columns
NCHUNK = HALF // FT    # 20
W_SLOW = 8000          # slow-EMA prepass window (as^8000 ~ 2.4e-4)
NPRE = W_SLOW // FT    # 4
EPS = np.float32(1e-6)

BUFS = 4
# columns of each flexible op placed on DVE (rest on GPSIMD)
SPLIT_S = 0        # s3 add: all GPSIMD
SPLIT_T1 = FT      # t1 mult: all DVE
SPLIT_OUT = 0      # out add: all GPSIMD


def _f32(v) -> np.float32:
    return np.float32(np.asarray(v).reshape(()))


# ---- custom DVE op: fused relu+EMA scan with ramp diagonalization ----

def _ema_ref(in0, in1, s0, s1, imm2):
    x = np.asarray(in0, np.float32)
    P = x.shape[0]
    xf = x.reshape(P, -1)
    Tn = xf.shape[1]
    r = np.asarray(in1, np.float32).reshape(P, -1)
    s0 = np.asarray(s0, np.float32).reshape(P, 1)
    u = np.maximum(xf, 0.0) * r
    u = s0 * np.float32(s1) + np.cumsum(u, axis=1, dtype=np.float32)
    g = np.float32(s1) ** np.arange(Tn, dtype=np.float32)
    return (u * g).astype(np.float32).reshape(x.shape)


def _register_dve_op(name, spec, subdim):
    for o in dve_ops.OPS:
        if o.name == name:
            return o
    opcode = dve_ops._CUSTOM_DVE_ROW_BASE + len(dve_ops.OPS)
    dve_ops._SUB_OPCODE_FOR_NAME[name] = opcode
    shas = {}
    for ver in ("v3", "v4"):
        uops = _dlower(spec, ver=ver)
        shas[ver] = _DveOpSpec(name=name, opcode=opcode, uops=uops,
                               rd1_en=_has_src1(spec)).sha(ver)
    op = dve_ops.DveOp(name, spec, subdim=subdim, uops_sha=shas)
    dve_ops.OPS.append(op)
    dve_ops.CUSTOM_DVE_SPECS[name] = spec
    return op


_geo = _DScan(_DAlu.MULTIPLY, _DZero, init=_DOne, _subdim_step=_C1)
_EMA_SPEC = _DSpec(
    body=_DScan(_DAlu.ADD, _drelu(_Src0) * _Src1,
                init=_DBin(_DAlu.MULTIPLY, _C0, _C1)) * _geo,
    reference=_ema_ref,
)
EMA_OP = _register_dve_op("RELU_EMA_RAMP_ANT", _EMA_SPEC, subdim=True)


def _recip1n_ref(in0, in1, s0, s1, imm2):
    u = (np.float32(s0) - np.asarray(in0, np.float32)).astype(np.float32)
    nx = (~u.view(np.int32)).view(np.float32)
    y0 = nx * np.float32(s1)
    return (y0 * (np.float32(imm2) - u * y0)).astype(np.float32)


# q ~= 1/(C0 - Src0): bitwise-not seed + one Chebyshev-tuned NR (~0.4% rel).
# Called with Src0 = -(Pf+Ps) and C0 = mh*eps, so q ~= 1/(mh*(M+eps)).
_ru = _DBin(_DAlu.SUBTRACT, _C0, _Src0)
_rnx = _DBin(_DAlu.BITWISE_NOT, _ru, _ru)
_ry0 = _rnx * _C1
_RECIP1N_SPEC = _DSpec(
    body=_ry0 * (_C2 - _ru * _ry0),
    reference=_recip1n_ref,
)
RECIP1N_OP = _register_dve_op("RECIP1N_BIAS_ANT", _RECIP1N_SPEC, subdim=False)
_CHEB0 = -0.23549792
_CHEB1 = 2.0017324


def _recip2n_ref(in0, in1, s0, s1, imm2):
    u = (np.float32(s0) - np.asarray(in0, np.float32)
         - np.asarray(in1, np.float32)).astype(np.float32)
    nx = (~u.view(np.int32)).view(np.float32)
    y0 = nx * np.float32(s1)
    return (y0 * (np.float32(imm2) - u * y0)).astype(np.float32)


# q ~= 1/(C0 - Src0 - Src1): consumes -Pf and -Ps directly, skipping the
# combined S tensor on the critical chain.
_ru2 = _DBin(_DAlu.SUBTRACT, _DBin(_DAlu.SUBTRACT, _C0, _Src0), _Src1)
_rnx2 = _DBin(_DAlu.BITWISE_NOT, _ru2, _ru2)
_ry02 = _rnx2 * _C1
_RECIP2N_SPEC = _DSpec(
    body=_ry02 * (_C2 - _ru2 * _ry02),
    reference=_recip2n_ref,
)
RECIP2N_OP = _register_dve_op("RECIP2N_BIAS_ANT", _RECIP2N_SPEC, subdim=False)


def _tt_split(nc, split, out, in0, in1, op):
    n = out.shape[1]
    if 0 < split < n:
        nc.vector.tensor_tensor(out[:, :split], in0[:, :split], in1[:, :split], op=op)
        nc.gpsimd.tensor_tensor(out[:, split:], in0[:, split:], in1[:, split:], op=op)
    elif split >= n:
        nc.vector.tensor_tensor(out[:], in0[:], in1[:], op=op)
    else:
        nc.gpsimd.tensor_tensor(out[:], in0[:], in1[:], op=op)


def _build(consts: dict, reps: int = 0):
    af = float(consts["af"]); as_ = float(consts["as"])
    Q = float(consts["Q"]); R = float(consts["R"])
    E = float(consts["E"])
    mh = float(consts["mh"])
    mh_ma = float(consts["mh_ma"]); mh_1ma = float(consts["mh_1ma"])

    nc = bacc.Bacc("TRN2", target_bir_lowering=False, debug=False, num_devices=B)
    x_d = nc.dram_tensor("x", [C, T], _dt, kind="ExternalInput")
    r_d = nc.dram_tensor("rmp", [128, 2 * FT], _dt, kind="ExternalInput")
    y_d = nc.dram_tensor("y", [C, T], _dt, kind="ExternalOutput")

    def sub3(ap):
        return ap.rearrange("p (s n) -> p s n", n=1)

    with tile.TileContext(nc) as tc, ExitStack() as ctx:
        cpool = ctx.enter_context(tc.tile_pool(name="consts", bufs=1))
        # cols: 0 init_f, 1 init_s, 2 zero, 5/6 scratch
        cons = cpool.tile([128, 8], _dt, tag="cons")
        nc.vector.memset(cons[:, 2:3], 0.0)
        init_f = cons[:, 0:1]
        init_s = cons[:, 1:2]

        # Decay ramps arrive precomputed from the host (negated: the EMA
        # outputs are -Pf/-Ps so every combine is an ADD).
        rpool = ctx.enter_context(tc.tile_pool(name="ramps", bufs=1))
        rdf = rpool.tile([128, FT], _dt, tag="rdf")
        rds = rpool.tile([128, FT], _dt, tag="rds")
        nc.sync.dma_start(rdf[:], r_d.ap()[:, 0:FT])
        nc.sync.dma_start(rds[:], r_d.ap()[:, FT:2 * FT])

        mpool = ctx.enter_context(tc.tile_pool(name="main", bufs=BUFS))

        def ema(out_ap, in_ap, ramp_ap, carry_ap, a):
            nc.vector._custom_dve(EMA_OP, out=out_ap, in0=sub3(in_ap),
                                  in1=ramp_ap, s0=carry_ap, s1=float(a))

        def body(_iv=None):
            # ---- prepass: half-1 scan initials from half-0 tail ----
            # A custom DVE op on a partial partition range is silently
            # skipped on HW — every ema() below must span all 128
            # partitions, so fill the unused half with zeros.
            pre_ps = None
            pre_pf = None
            for k in range(NPRE):
                lo = HALF - W_SLOW + k * FT
                xp = mpool.tile([128, FT], _dt, tag="x")
                nc.gpsimd.memset(xp[0:64, :], 0.0)
                nc.sync.dma_start(xp[64:128, :], x_d.ap()[:, lo:lo + FT])
                po = mpool.tile([128, FT], _dt, tag="ps")
                carry = cons[:, 2:3] if pre_ps is None \
                    else pre_ps[:, FT - 1:FT]
                ema(po[:], xp[:], rds[:], carry, as_)
                pre_ps = po
                if k == NPRE - 1:
                    fo = mpool.tile([128, FT], _dt, tag="pf")
                    ema(fo[:], xp[:], rdf[:], cons[:, 2:3], af)
                    pre_pf = fo
            nc.scalar.copy(init_f[64:128, :], pre_pf[64:128, FT - 1:FT])
            nc.scalar.copy(init_s[64:128, :], pre_ps[64:128, FT - 1:FT])

            # ---- main streaming loop (all values in NEGATED EMA domain) ----
            prev_pf = None
            prev_ps = None
            for j in range(NCHUNK):
                sl = slice(j * FT, (j + 1) * FT)
                sh = slice(HALF + j * FT, HALF + (j + 1) * FT)
                xt = mpool.tile([128, FT], _dt, tag="x")
                # 2D loads spread over all 16 SDMA engines; 3D would not.
                nc.sync.dma_start(xt[0:64, :], x_d.ap()[:, sl])
                nc.sync.dma_start(xt[64:128, :], x_d.ap()[:, sh])

                if j == 0:
                    # init = -mh_ma*relu(x0): relu then negate (tiny [64,1])
                    nc.scalar.activation(cons[0:64, 5:6], xt[0:64, 0:1],
                                         _Act.Relu, scale=mh_ma)
                    nc.scalar.activation(init_f[0:64, :], cons[0:64, 5:6],
                                         _Act.Copy, scale=-1.0)
                    nc.scalar.activation(cons[0:64, 6:7], xt[0:64, 0:1],
                                         _Act.Relu, scale=mh_1ma)
                    nc.scalar.activation(init_s[0:64, :], cons[0:64, 6:7],
                                         _Act.Copy, scale=-1.0)

                pfn = mpool.tile([128, FT], _dt, tag="pf")  # -Pf
                cf = init_f if j == 0 else prev_pf[:, FT - 1:FT]
                ema(pfn[:], xt[:], rdf[:], cf, af)
                psn = mpool.tile([128, FT], _dt, tag="ps")  # -Ps
                cs = init_s if j == 0 else prev_ps[:, FT - 1:FT]
                ema(psn[:], xt[:], rds[:], cs, as_)
                prev_pf, prev_ps = pfn, psn

                # q straight from the two EMA outputs (critical chain);
                # s3n on GPSIMD in parallel (only 'out' needs it)
                q = mpool.tile([128, FT], _dt, tag="q")
                nc.vector._custom_dve(RECIP2N_OP, out=q[:], in0=pfn[:],
                                      in1=psn[:], s0=float(E), s1=_CHEB0,
                                      imm2=_CHEB1)

                s3n = mpool.tile([128, FT], _dt, tag="s3")  # -(Pf+Ps)
                _tt_split(nc, SPLIT_S, s3n, pfn, psn, _A.add)

                xr = xt  # in-place: x dead after the EMAs read it
                nc.scalar.activation(xr[:], xt[:], _Act.Relu, scale=mh)

                r2 = q  # in-place: r2 = q + 1  [ACT]
                nc.scalar.activation(r2[:], q[:], _Act.Copy, bias=1.0)

                t1 = mpool.tile([128, FT], _dt, tag="t1")
                _tt_split(nc, SPLIT_T1, t1, r2, xr, _A.mult)

                o = xr  # xr dead after t1; out = t1 + (-(Pf+Ps)) (drops -E)
                _tt_split(nc, SPLIT_OUT, o, t1, s3n, _A.add)

                nc.scalar.dma_start(y_d.ap()[:, sl], o[0:64, :])
                nc.scalar.dma_start(y_d.ap()[:, sh], o[64:128, :])

        if reps > 0:
            for _ in range(reps):
                body()
        else:
            body()

    nc.compile()
    return nc


_CACHE: dict = {}


def _get_nc(consts: dict, reps: int = 0):
    key = (tuple(sorted(consts.items())), reps, BUFS, SPLIT_S, SPLIT_T1,
           SPLIT_OUT, FT, W_SLOW)
    if key not in _CACHE:
        _CACHE[key] = _build(consts, reps)
    return _CACHE[key]


def _consts_from_inputs(mu_fast, mu_slow, mix_weight_adapt, mix_weight_hp) -> dict:
    mu_f = _f32(mu_fast)
    mu_s = _f32(mu_slow)
    one = np.float32(1.0)
    ma = np.float32(one / (one + np.exp(np.float32(-mix_weight_adapt))))
    mh = np.float32(one / (one + np.exp(np.float32(-mix_weight_hp))))
    af = one - mu_f
    as_ = one - mu_s
    d = dict(
        af=float(af),
        Q=float(np.float32(mh * ma * mu_f)),
        R=float(np.float32(mh * (one - ma) * mu_s)),
        E=float(np.float32(mh * EPS)),
        mh=float(mh),
        mh_ma=float(np.float32(mh * ma)),
        mh_1ma=float(np.float32(mh * (one - ma))),
    )
    d["as"] = float(as_)
    return d


def _ramps_from_consts(consts: dict) -> np.ndarray:
    """[128, 2*FT] negated decay ramps: col k -> -Q*af^-k | -R*as^-k."""
    k = np.arange(FT, dtype=np.float64)
    rdf = (-consts["Q"] * np.float64(consts["af"]) ** -k).astype(np.float32)
    rds = (-consts["R"] * np.float64(consts["as"]) ** -k).astype(np.float32)
    return np.ascontiguousarray(
        np.broadcast_to(np.concatenate([rdf, rds]), (128, 2 * FT)))


def _make_in_maps(consts: dict, x: np.ndarray) -> list:
    rmp = _ramps_from_consts(consts)
    return [{"x": np.ascontiguousarray(x[b]), "rmp": rmp} for b in range(B)]


def kernel(x, mu_fast, mu_slow, mix_weight_adapt, mix_weight_hp):
    x = np.asarray(x, dtype=np.float32)
    assert x.shape == (B, C, T), x.shape
    consts = _consts_from_inputs(mu_fast, mu_slow, mix_weight_adapt, mix_weight_hp)
    nc = _get_nc(consts)
    res = run_bass_kernel_spmd(nc, _make_in_maps(consts, x),
                               core_ids=list(range(B)))
    return np.stack([res.results[b]["y"] for b in range(B)], axis=0)


if __name__ == "__main__":
    rng = np.random.default_rng(0)
    import math
    FS = 16000.0
    x = rng.standard_normal((B, C, T), dtype=np.float32)
    out = kernel(
        x,
        np.float32(1.0 - math.exp(-1.0 / (FS * 2.0 / 1000.0))),
        np.float32(1.0 - math.exp(-1.0 / (FS * 60.0 / 1000.0))),
        np.float32(0.5),
        np.float32(0.3),
    )
    print(out.shape, out.dtype, np.isfinite(out).all())


# revision 32
# speedup vs baseline: 1.1680x; 1.1680x over previous
"""Trainium2 Bass kernel for DualTimeConstantHighPassMixAdaptation.

Math (reference):
    xr = relu(x)
    Mf[t] = (1-mu_f)*Mf[t-1] + mu_f*xr[t],  Mf[0] = xr[0]   (same for Ms)
    M  = ma*Mf + (1-ma)*Ms,   ma = sigmoid(mix_weight_adapt)
    out = xr/(eps+M) + mh*(xr - M),         mh = sigmoid(mix_weight_hp)

Kernel formulation:
    Pf = mh*ma*Mf, Ps = mh*(1-ma)*Ms, S = Pf + Ps + mh*eps = mh*(M+eps)
    q  = 1/S ;  t1 = (q+1)*mh*xr ;  out = t1 - S   (drops +mh*eps ~ 5.7e-7)

Each EMA chunk is ONE custom DVE op (bubble-free, ~1 elem/cycle/lane),
using the exponential-ramp diagonalization:
    Pf[t] = af^t * (af*carry + sum_k (Q*af^-k) * relu(x[k]))
body = scan(ADD, relu(Src0)*Src1, init=C0*C1) * geo(C1), where Src1 is a
precomputed decay ramp (constants folded in) and geo is a per-element
geometric ramp from the subdim-step scan (pages of 1).

Sharding: core b <- batch b (64 channel lanes). In-core the 64 lanes are
split into two time halves -> 128 partitions x 32000 samples. Half-1 rows
get their scan initial state from a prepass that re-scans the last
W samples of half-0 (EMA decay bounds the truncation error ~a^W).

Input loads / output stores are pairs of 2D HWDGE DMAs (a 3D access
pattern lands on only 2 of 16 SDMA engines; 2D spreads across all 16).
"""

import sys

for _p in ("/opt/trn_rl_repo", "/root/.axon_site/_ro/trn_rl_repo"):
    if _p not in sys.path:
        sys.path.insert(0, _p)

from contextlib import ExitStack

import numpy as np

import concourse.bacc as bacc
import concourse.tile as tile
from concourse import mybir
from concourse.bass_utils import run_bass_kernel_spmd

from concourse import dve_ops
from concourse.dve_spec import (AluOp as _DAlu, Bin as _DBin, Scan as _DScan,
                                Spec as _DSpec, Src0 as _Src0, Src1 as _Src1,
                                C0 as _C0, C1 as _C1, C2 as _C2, Zero as _DZero,
                                One as _DOne, relu as _drelu, lower as _dlower,
                                _has_src1)
from concourse.dve_uop import DveOpSpec as _DveOpSpec

_dt = mybir.dt.float32
_A = mybir.AluOpType
_Act = mybir.ActivationFunctionType

# Problem geometry (hardcoded per spec).
B, C, T = 8, 64, 64000
HALF = T // 2          # 32000
FT = 2000              # chunk columns
NCHUNK = HALF // FT    # 20
W_SLOW = 8000          # slow-EMA prepass window (as^8000 ~ 2.4e-4)
NPRE = W_SLOW // FT    # 4
EPS = np.float32(1e-6)

BUFS = 3
# columns of each flexible op placed on DVE (rest on GPSIMD)
SPLIT_S = 0        # s3 add: all GPSIMD
SPLIT_T1 = FT      # t1 mult: all DVE
SPLIT_OUT = 0      # out add: all GPSIMD


def _f32(v) -> np.float32:
    return np.float32(np.asarray(v).reshape(()))


# ---- custom DVE op: fused relu+EMA scan with ramp diagonalization ----

def _ema_ref(in0, in1, s0, s1, imm2):
    x = np.asarray(in0, np.float32)
    P = x.shape[0]
    xf = x.reshape(P, -1)
    Tn = xf.shape[1]
    r = np.asarray(in1, np.float32).reshape(P, -1)
    s0 = np.asarray(s0, np.float32).reshape(P, 1)
    u = np.maximum(xf, 0.0) * r
    u = s0 * np.float32(s1) + np.cumsum(u, axis=1, dtype=np.float32)
    g = np.float32(s1) ** np.arange(Tn, dtype=np.float32)
    return (u * g).astype(np.float32).reshape(x.shape)


def _register_dve_op(name, spec, subdim):
    for o in dve_ops.OPS:
        if o.name == name:
            return o
    opcode = dve_ops._CUSTOM_DVE_ROW_BASE + len(dve_ops.OPS)
    dve_ops._SUB_OPCODE_FOR_NAME[name] = opcode
    shas = {}
    for ver in ("v3", "v4"):
        uops = _dlower(spec, ver=ver)
        shas[ver] = _DveOpSpec(name=name, opcode=opcode, uops=uops,
                               rd1_en=_has_src1(spec)).sha(ver)
    op = dve_ops.DveOp(name, spec, subdim=subdim, uops_sha=shas)
    dve_ops.OPS.append(op)
    dve_ops.CUSTOM_DVE_SPECS[name] = spec
    return op


_geo = _DScan(_DAlu.MULTIPLY, _DZero, init=_DOne, _subdim_step=_C1)
_EMA_SPEC = _DSpec(
    body=_DScan(_DAlu.ADD, _drelu(_Src0) * _Src1,
                init=_DBin(_DAlu.MULTIPLY, _C0, _C1)) * _geo,
    reference=_ema_ref,
)
EMA_OP = _register_dve_op("RELU_EMA_RAMP_ANT", _EMA_SPEC, subdim=True)


def _recip1n_ref(in0, in1, s0, s1, imm2):
    u = (np.float32(s0) - np.asarray(in0, np.float32)).astype(np.float32)
    nx = (~u.view(np.int32)).view(np.float32)
    y0 = nx * np.float32(s1)
    return (y0 * (np.float32(imm2) - u * y0)).astype(np.float32)


# q ~= 1/(C0 - Src0): bitwise-not seed + one Chebyshev-tuned NR (~0.4% rel).
# Called with Src0 = -(Pf+Ps) and C0 = mh*eps, so q ~= 1/(mh*(M+eps)).
_ru = _DBin(_DAlu.SUBTRACT, _C0, _Src0)
_rnx = _DBin(_DAlu.BITWISE_NOT, _ru, _ru)
_ry0 = _rnx * _C1
_RECIP1N_SPEC = _DSpec(
    body=_ry0 * (_C2 - _ru * _ry0),
    reference=_recip1n_ref,
)
RECIP1N_OP = _register_dve_op("RECIP1N_BIAS_ANT", _RECIP1N_SPEC, subdim=False)
_CHEB0 = -0.23549792
_CHEB1 = 2.0017324


def _recip2n_ref(in0, in1, s0, s1, imm2):
    u = (np.float32(s0) - np.asarray(in0, np.float32)
         - np.asarray(in1, np.float32)).astype(np.float32)
    nx = (~u.view(np.int32)).view(np.float32)
    y0 = nx * np.float32(s1)
    return (y0 * (np.float32(imm2) - u * y0)).astype(np.float32)


# q ~= 1/(C0 - Src0 - Src1): consumes -Pf and -Ps directly, skipping the
# combined S tensor on the critical chain.
_ru2 = _DBin(_DAlu.SUBTRACT, _DBin(_DAlu.SUBTRACT, _C0, _Src0), _Src1)
_rnx2 = _DBin(_DAlu.BITWISE_NOT, _ru2, _ru2)
_ry02 = _rnx2 * _C1
_RECIP2N_SPEC = _DSpec(
    body=_ry02 * (_C2 - _ru2 * _ry02),
    reference=_recip2n_ref,
)
RECIP2N_OP = _register_dve_op("RECIP2N_BIAS_ANT", _RECIP2N_SPEC, subdim=False)


def _tt_split(nc, split, out, in0, in1, op):
    n = out.shape[1]
    if 0 < split < n:
        nc.vector.tensor_tensor(out[:, :split], in0[:, :split], in1[:, :split], op=op)
        nc.gpsimd.tensor_tensor(out[:, split:], in0[:, split:], in1[:, split:], op=op)
    elif split >= n:
        nc.vector.tensor_tensor(out[:], in0[:], in1[:], op=op)
    else:
        nc.gpsimd.tensor_tensor(out[:], in0[:], in1[:], op=op)


def _build(consts: dict, reps: int = 0):
    af = float(consts["af"]); as_ = float(consts["as"])
    Q = float(consts["Q"]); R = float(consts["R"])
    E = float(consts["E"])
    mh = float(consts["mh"])
    mh_ma = float(consts["mh_ma"]); mh_1ma = float(consts["mh_1ma"])

    nc = bacc.Bacc("TRN2", target_bir_lowering=False, debug=False, num_devices=B)
    x_d = nc.dram_tensor("x", [C, T], _dt, kind="ExternalInput")
    r_d = nc.dram_tensor("rmp", [128, 2 * FT], _dt, kind="ExternalInput")
    y_d = nc.dram_tensor("y", [C, T], _dt, kind="ExternalOutput")

    def sub3(ap):
        return ap.rearrange("p (s n) -> p s n", n=1)

    with tile.TileContext(nc) as tc, ExitStack() as ctx:
        cpool = ctx.enter_context(tc.tile_pool(name="consts", bufs=1))
        # cols: 0 init_f, 1 init_s, 2 zero, 5/6 scratch
        cons = cpool.tile([128, 8], _dt, tag="cons")
        nc.vector.memset(cons[:, 2:3], 0.0)
        init_f = cons[:, 0:1]
        init_s = cons[:, 1:2]

        # Decay ramps arrive precomputed from the host (negated: the EMA
        # outputs are -Pf/-Ps so every combine is an ADD).
        rpool = ctx.enter_context(tc.tile_pool(name="ramps", bufs=1))
        rdf = rpool.tile([128, FT], _dt, tag="rdf")
        rds = rpool.tile([128, FT], _dt, tag="rds")
        nc.sync.dma_start(rdf[:], r_d.ap()[:, 0:FT])
        nc.sync.dma_start(rds[:], r_d.ap()[:, FT:2 * FT])

        mpool = ctx.enter_context(tc.tile_pool(name="main", bufs=BUFS))

        def ema(out_ap, in_ap, ramp_ap, carry_ap, a):
            nc.vector._custom_dve(EMA_OP, out=out_ap, in0=sub3(in_ap),
                                  in1=ramp_ap, s0=carry_ap, s1=float(a))

        def body(_iv=None):
            # ---- prepass: half-1 scan initials from half-0 tail ----
            # A custom DVE op on a partial partition range is silently
            # skipped on HW — every ema() below must span all 128
            # partitions, so fill the unused half with zeros.
            pre_ps = None
            pre_pf = None
            for k in range(NPRE):
                lo = HALF - W_SLOW + k * FT
                xp = mpool.tile([128, FT], _dt, tag="x")
                nc.gpsimd.memset(xp[0:64, :], 0.0)
                nc.sync.dma_start(xp[64:128, :], x_d.ap()[:, lo:lo + FT])
                po = mpool.tile([128, FT], _dt, tag="ps")
                carry = cons[:, 2:3] if pre_ps is None \
                    else pre_ps[:, FT - 1:FT]
                ema(po[:], xp[:], rds[:], carry, as_)
                pre_ps = po
                if k == NPRE - 1:
                    fo = mpool.tile([128, FT], _dt, tag="pf")
                    ema(fo[:], xp[:], rdf[:], cons[:, 2:3], af)
                    pre_pf = fo
            nc.scalar.copy(init_f[64:128, :], pre_pf[64:128, FT - 1:FT])
            nc.scalar.copy(init_s[64:128, :], pre_ps[64:128, FT - 1:FT])

            # ---- main streaming loop (all values in NEGATED EMA domain) ----
            prev_pf = None
            prev_ps = None
            for j in range(NCHUNK):
                sl = slice(j * FT, (j + 1) * FT)
                sh = slice(HALF + j * FT, HALF + (j + 1) * FT)
                xt = mpool.tile([128, FT], _dt, tag="x")
                # 2D loads spread over all 16 SDMA engines; 3D would not.
                nc.sync.dma_start(xt[0:64, :], x_d.ap()[:, sl])
                nc.sync.dma_start(xt[64:128, :], x_d.ap()[:, sh])

                if j == 0:
                    # init = -mh_ma*relu(x0): relu then negate (tiny [64,1])
                    nc.scalar.activation(cons[0:64, 5:6], xt[0:64, 0:1],
                                         _Act.Relu, scale=mh_ma)
                    nc.scalar.activation(init_f[0:64, :], cons[0:64, 5:6],
                                         _Act.Copy, scale=-1.0)
                    nc.scalar.activation(cons[0:64, 6:7], xt[0:64, 0:1],
                                         _Act.Relu, scale=mh_1ma)
                    nc.scalar.activation(init_s[0:64, :], cons[0:64, 6:7],
                                         _Act.Copy, scale=-1.0)

                pfn = mpool.tile([128, FT], _dt, tag="pf")  # -Pf
                cf = init_f if j == 0 else prev_pf[:, FT - 1:FT]
                ema(pfn[:], xt[:], rdf[:], cf, af)
                psn = mpool.tile([128, FT], _dt, tag="ps")  # -Ps
                cs = init_s if j == 0 else prev_ps[:, FT - 1:FT]
                ema(psn[:], xt[:], rds[:], cs, as_)
                prev_pf, prev_ps = pfn, psn

                # q straight from the two EMA outputs (critical chain);
                # s3n on GPSIMD in parallel (only 'out' needs it)
                q = mpool.tile([128, FT], _dt, tag="q")
                nc.vector._custom_dve(RECIP2N_OP, out=q[:], in0=pfn[:],
                                      in1=psn[:], s0=float(E), s1=_CHEB0,
                                      imm2=_CHEB1)

                s3n = mpool.tile([128, FT], _dt, tag="s3")  # -(Pf+Ps)
                _tt_split(nc, SPLIT_S, s3n, pfn, psn, _A.add)

                xr = mpool.tile([128, FT], _dt, tag="xr")
                nc.scalar.activation(xr[:], xt[:], _Act.Relu, scale=mh)

                r2 = q  # in-place: r2 = q + 1  [ACT]
                nc.scalar.activation(r2[:], q[:], _Act.Copy, bias=1.0)

                t1 = mpool.tile([128, FT], _dt, tag="t1")
                _tt_split(nc, SPLIT_T1, t1, r2, xr, _A.mult)

                o = xr  # xr dead after t1; out = t1 + (-(Pf+Ps)) (drops -E)
                _tt_split(nc, SPLIT_OUT, o, t1, s3n, _A.add)

                nc.scalar.dma_start(y_d.ap()[:, sl], o[0:64, :])
                nc.scalar.dma_start(y_d.ap()[:, sh], o[64:128, :])

        if reps > 0:
            for _ in range(reps):
                body()
        else:
            body()

    nc.compile()
    return nc


_CACHE: dict = {}


def _get_nc(consts: dict, reps: int = 0):
    key = (tuple(sorted(consts.items())), reps, BUFS, SPLIT_S, SPLIT_T1,
           SPLIT_OUT, FT, W_SLOW)
    if key not in _CACHE:
        _CACHE[key] = _build(consts, reps)
    return _CACHE[key]


def _consts_from_inputs(mu_fast, mu_slow, mix_weight_adapt, mix_weight_hp) -> dict:
    mu_f = _f32(mu_fast)
    mu_s = _f32(mu_slow)
    one = np.float32(1.0)
    ma = np.float32(one / (one + np.exp(np.float32(-mix_weight_adapt))))
    mh = np.float32(one / (one + np.exp(np.float32(-mix_weight_hp))))
    af = one - mu_f
    as_ = one - mu_s
    d = dict(
        af=float(af),
        Q=float(np.float32(mh * ma * mu_f)),
        R=float(np.float32(mh * (one - ma) * mu_s)),
        E=float(np.float32(mh * EPS)),
        mh=float(mh),
        mh_ma=float(np.float32(mh * ma)),
        mh_1ma=float(np.float32(mh * (one - ma))),
    )
    d["as"] = float(as_)
    return d


def _ramps_from_consts(consts: dict) -> np.ndarray:
    """[128, 2*FT] negated decay ramps: col k -> -Q*af^-k | -R*as^-k."""
    k = np.arange(FT, dtype=np.float64)
    rdf = (-consts["Q"] * np.float64(consts["af"]) ** -k).astype(np.float32)
    rds = (-consts["R"] * np.float64(consts["as"]) ** -k).astype(np.float32)
    return np.ascontiguousarray(
        np.broadcast_to(np.concatenate([rdf, rds]), (128, 2 * FT)))


def _make_in_maps(consts: dict, x: np.ndarray) -> list:
    rmp = _ramps_from_consts(consts)
    return [{"x": np.ascontiguousarray(x[b]), "rmp": rmp} for b in range(B)]


def kernel(x, mu_fast, mu_slow, mix_weight_adapt, mix_weight_hp):
    x = np.asarray(x, dtype=np.float32)
    assert x.shape == (B, C, T), x.shape
    consts = _consts_from_inputs(mu_fast, mu_slow, mix_weight_adapt, mix_weight_hp)
    nc = _get_nc(consts)
    res = run_bass_kernel_spmd(nc, _make_in_maps(consts, x),
                               core_ids=list(range(B)))
    return np.stack([res.results[b]["y"] for b in range(B)], axis=0)


if __name__ == "__main__":
    rng = np.random.default_rng(0)
    import math
    FS = 16000.0
    x = rng.standard_normal((B, C, T), dtype=np.float32)
    out = kernel(
        x,
        np.float32(1.0 - math.exp(-1.0 / (FS * 2.0 / 1000.0))),
        np.float32(1.0 - math.exp(-1.0 / (FS * 60.0 / 1000.0))),
        np.float32(0.5),
        np.float32(0.3),
    )
    print(out.shape, out.dtype, np.isfinite(out).all())


# revision 34
# speedup vs baseline: 1.1926x; 1.0211x over previous
"""Trainium2 Bass kernel for DualTimeConstantHighPassMixAdaptation.

Math (reference):
    xr = relu(x)
    Mf[t] = (1-mu_f)*Mf[t-1] + mu_f*xr[t],  Mf[0] = xr[0]   (same for Ms)
    M  = ma*Mf + (1-ma)*Ms,   ma = sigmoid(mix_weight_adapt)
    out = xr/(eps+M) + mh*(xr - M),         mh = sigmoid(mix_weight_hp)

Kernel formulation:
    Pf = mh*ma*Mf, Ps = mh*(1-ma)*Ms, S = Pf + Ps + mh*eps = mh*(M+eps)
    q  = 1/S ;  t1 = (q+1)*mh*xr ;  out = t1 - S   (drops +mh*eps ~ 5.7e-7)

Each EMA chunk is ONE custom DVE op (bubble-free, ~1 elem/cycle/lane),
using the exponential-ramp diagonalization:
    Pf[t] = af^t * (af*carry + sum_k (Q*af^-k) * relu(x[k]))
body = scan(ADD, relu(Src0)*Src1, init=C0*C1) * geo(C1), where Src1 is a
precomputed decay ramp (constants folded in) and geo is a per-element
geometric ramp from the subdim-step scan (pages of 1).

Sharding: core b <- batch b (64 channel lanes). In-core the 64 lanes are
split into two time halves -> 128 partitions x 32000 samples. Half-1 rows
get their scan initial state from a prepass that re-scans the last
W samples of half-0 (EMA decay bounds the truncation error ~a^W).

Input loads / output stores are pairs of 2D HWDGE DMAs (a 3D access
pattern lands on only 2 of 16 SDMA engines; 2D spreads across all 16).
"""

import sys

for _p in ("/opt/trn_rl_repo", "/root/.axon_site/_ro/trn_rl_repo"):
    if _p not in sys.path:
        sys.path.insert(0, _p)

from contextlib import ExitStack

import numpy as np

import concourse.bacc as bacc
import concourse.tile as tile
from concourse import mybir
from concourse.bass_utils import run_bass_kernel_spmd

from concourse import dve_ops
from concourse.dve_spec import (AluOp as _DAlu, Bin as _DBin, Scan as _DScan,
                                Spec as _DSpec, Src0 as _Src0, Src1 as _Src1,
                                C0 as _C0, C1 as _C1, C2 as _C2, Zero as _DZero,
                                One as _DOne, relu as _drelu, lower as _dlower,
                                _has_src1)
from concourse.dve_uop import DveOpSpec as _DveOpSpec

_dt = mybir.dt.float32
_A = mybir.AluOpType
_Act = mybir.ActivationFunctionType

# Problem geometry (hardcoded per spec).
B, C, T = 8, 64, 64000
HALF = T // 2          # 32000
FT = 2000              # chunk columns
NCHUNK = HALF // FT    # 20
W_SLOW = 8000          # slow-EMA prepass window (as^8000 ~ 2.4e-4)
NPRE = W_SLOW // FT    # 4
EPS = np.float32(1e-6)

BUFS = 3
# columns of each flexible op placed on DVE (rest on GPSIMD)
SPLIT_S = 0        # s3 add: all GPSIMD
SPLIT_T1 = FT      # t1 mult: all DVE
SPLIT_OUT = 0      # out add: all GPSIMD


def _f32(v) -> np.float32:
    return np.float32(np.asarray(v).reshape(()))


# ---- custom DVE op: fused relu+EMA scan with ramp diagonalization ----

def _ema_ref(in0, in1, s0, s1, imm2):
    x = np.asarray(in0, np.float32)
    P = x.shape[0]
    xf = x.reshape(P, -1)
    Tn = xf.shape[1]
    r = np.asarray(in1, np.float32).reshape(P, -1)
    s0 = np.asarray(s0, np.float32).reshape(P, 1)
    u = np.maximum(xf, 0.0) * r
    u = s0 * np.float32(s1) + np.cumsum(u, axis=1, dtype=np.float32)
    g = np.float32(s1) ** np.arange(Tn, dtype=np.float32)
    return (u * g).astype(np.float32).reshape(x.shape)


def _register_dve_op(name, spec, subdim):
    for o in dve_ops.OPS:
        if o.name == name:
            return o
    opcode = dve_ops._CUSTOM_DVE_ROW_BASE + len(dve_ops.OPS)
    dve_ops._SUB_OPCODE_FOR_NAME[name] = opcode
    shas = {}
    for ver in ("v3", "v4"):
        uops = _dlower(spec, ver=ver)
        shas[ver] = _DveOpSpec(name=name, opcode=opcode, uops=uops,
                               rd1_en=_has_src1(spec)).sha(ver)
    op = dve_ops.DveOp(name, spec, subdim=subdim, uops_sha=shas)
    dve_ops.OPS.append(op)
    dve_ops.CUSTOM_DVE_SPECS[name] = spec
    return op


_geo = _DScan(_DAlu.MULTIPLY, _DZero, init=_DOne, _subdim_step=_C1)
_EMA_SPEC = _DSpec(
    body=_DScan(_DAlu.ADD, _drelu(_Src0) * _Src1,
                init=_DBin(_DAlu.MULTIPLY, _C0, _C1)) * _geo,
    reference=_ema_ref,
)
EMA_OP = _register_dve_op("RELU_EMA_RAMP_ANT", _EMA_SPEC, subdim=True)


def _recip1n_ref(in0, in1, s0, s1, imm2):
    u = (np.float32(s0) - np.asarray(in0, np.float32)).astype(np.float32)
    nx = (~u.view(np.int32)).view(np.float32)
    y0 = nx * np.float32(s1)
    return (y0 * (np.float32(imm2) - u * y0)).astype(np.float32)


# q ~= 1/(C0 - Src0): bitwise-not seed + one Chebyshev-tuned NR (~0.4% rel).
# Called with Src0 = -(Pf+Ps) and C0 = mh*eps, so q ~= 1/(mh*(M+eps)).
_ru = _DBin(_DAlu.SUBTRACT, _C0, _Src0)
_rnx = _DBin(_DAlu.BITWISE_NOT, _ru, _ru)
_ry0 = _rnx * _C1
_RECIP1N_SPEC = _DSpec(
    body=_ry0 * (_C2 - _ru * _ry0),
    reference=_recip1n_ref,
)
RECIP1N_OP = _register_dve_op("RECIP1N_BIAS_ANT", _RECIP1N_SPEC, subdim=False)
_CHEB0 = -0.23549792
_CHEB1 = 2.0017324


def _recip2n_ref(in0, in1, s0, s1, imm2):
    u = (np.float32(s0) - np.asarray(in0, np.float32)
         - np.asarray(in1, np.float32)).astype(np.float32)
    nx = (~u.view(np.int32)).view(np.float32)
    y0 = nx * np.float32(s1)
    return (y0 * (np.float32(imm2) - u * y0)).astype(np.float32)


# r2 ~= 1 + 1/(C0 - Src0 - Src1): consumes -Pf and -Ps directly, skipping
# both the combined S tensor and the ACT (+1) hop on the critical chain.
_ru2 = _DBin(_DAlu.SUBTRACT, _DBin(_DAlu.SUBTRACT, _C0, _Src0), _Src1)
_rnx2 = _DBin(_DAlu.BITWISE_NOT, _ru2, _ru2)
_ry02 = _rnx2 * _C1
_RECIP2N_SPEC = _DSpec(
    body=_ry02 * (_C2 - _ru2 * _ry02) + _DOne,
    reference=lambda *a: _recip2n_ref(*a) + np.float32(1.0),
)
RECIP2N_OP = _register_dve_op("RECIP2P1_BIAS_ANT", _RECIP2N_SPEC, subdim=False)


def _tt_split(nc, split, out, in0, in1, op):
    n = out.shape[1]
    if 0 < split < n:
        nc.vector.tensor_tensor(out[:, :split], in0[:, :split], in1[:, :split], op=op)
        nc.gpsimd.tensor_tensor(out[:, split:], in0[:, split:], in1[:, split:], op=op)
    elif split >= n:
        nc.vector.tensor_tensor(out[:], in0[:], in1[:], op=op)
    else:
        nc.gpsimd.tensor_tensor(out[:], in0[:], in1[:], op=op)


def _build(consts: dict, reps: int = 0):
    af = float(consts["af"]); as_ = float(consts["as"])
    Q = float(consts["Q"]); R = float(consts["R"])
    E = float(consts["E"])
    mh = float(consts["mh"])
    mh_ma = float(consts["mh_ma"]); mh_1ma = float(consts["mh_1ma"])

    nc = bacc.Bacc("TRN2", target_bir_lowering=False, debug=False, num_devices=B)
    x_d = nc.dram_tensor("x", [C, T], _dt, kind="ExternalInput")
    r_d = nc.dram_tensor("rmp", [128, 2 * FT], _dt, kind="ExternalInput")
    y_d = nc.dram_tensor("y", [C, T], _dt, kind="ExternalOutput")

    def sub3(ap):
        return ap.rearrange("p (s n) -> p s n", n=1)

    with tile.TileContext(nc) as tc, ExitStack() as ctx:
        cpool = ctx.enter_context(tc.tile_pool(name="consts", bufs=1))
        # cols: 0 init_f, 1 init_s, 2 zero, 5/6 scratch
        cons = cpool.tile([128, 8], _dt, tag="cons")
        nc.vector.memset(cons[:, 2:3], 0.0)
        init_f = cons[:, 0:1]
        init_s = cons[:, 1:2]

        # Decay ramps arrive precomputed from the host (negated: the EMA
        # outputs are -Pf/-Ps so every combine is an ADD).
        rpool = ctx.enter_context(tc.tile_pool(name="ramps", bufs=1))
        rdf = rpool.tile([128, FT], _dt, tag="rdf")
        rds = rpool.tile([128, FT], _dt, tag="rds")
        nc.sync.dma_start(rdf[:], r_d.ap()[:, 0:FT])
        nc.sync.dma_start(rds[:], r_d.ap()[:, FT:2 * FT])

        mpool = ctx.enter_context(tc.tile_pool(name="main", bufs=BUFS))

        def ema(out_ap, in_ap, ramp_ap, carry_ap, a):
            nc.vector._custom_dve(EMA_OP, out=out_ap, in0=sub3(in_ap),
                                  in1=ramp_ap, s0=carry_ap, s1=float(a))

        def body(_iv=None):
            # ---- prepass: half-1 scan initials from half-0 tail ----
            # A custom DVE op on a partial partition range is silently
            # skipped on HW — every ema() below must span all 128
            # partitions, so fill the unused half with zeros.
            pre_ps = None
            pre_pf = None
            for k in range(NPRE):
                lo = HALF - W_SLOW + k * FT
                xp = mpool.tile([128, FT], _dt, tag="x")
                nc.gpsimd.memset(xp[0:64, :], 0.0)
                nc.sync.dma_start(xp[64:128, :], x_d.ap()[:, lo:lo + FT])
                po = mpool.tile([128, FT], _dt, tag="ps")
                carry = cons[:, 2:3] if pre_ps is None \
                    else pre_ps[:, FT - 1:FT]
                ema(po[:], xp[:], rds[:], carry, as_)
                pre_ps = po
                if k == NPRE - 1:
                    fo = mpool.tile([128, FT], _dt, tag="pf")
                    ema(fo[:], xp[:], rdf[:], cons[:, 2:3], af)
                    pre_pf = fo
            nc.scalar.copy(init_f[64:128, :], pre_pf[64:128, FT - 1:FT])
            nc.scalar.copy(init_s[64:128, :], pre_ps[64:128, FT - 1:FT])

            # ---- main streaming loop (all values in NEGATED EMA domain) ----
            prev_pf = None
            prev_ps = None
            for j in range(NCHUNK):
                sl = slice(j * FT, (j + 1) * FT)
                sh = slice(HALF + j * FT, HALF + (j + 1) * FT)
                xt = mpool.tile([128, FT], _dt, tag="x")
                # 2D loads spread over all 16 SDMA engines; 3D would not.
                nc.sync.dma_start(xt[0:64, :], x_d.ap()[:, sl])
                nc.sync.dma_start(xt[64:128, :], x_d.ap()[:, sh])

                if j == 0:
                    # init = -mh_ma*relu(x0): relu then negate (tiny [64,1])
                    nc.scalar.activation(cons[0:64, 5:6], xt[0:64, 0:1],
                                         _Act.Relu, scale=mh_ma)
                    nc.scalar.activation(init_f[0:64, :], cons[0:64, 5:6],
                                         _Act.Copy, scale=-1.0)
                    nc.scalar.activation(cons[0:64, 6:7], xt[0:64, 0:1],
                                         _Act.Relu, scale=mh_1ma)
                    nc.scalar.activation(init_s[0:64, :], cons[0:64, 6:7],
                                         _Act.Copy, scale=-1.0)

                pfn = mpool.tile([128, FT], _dt, tag="pf")  # -Pf
                cf = init_f if j == 0 else prev_pf[:, FT - 1:FT]
                ema(pfn[:], xt[:], rdf[:], cf, af)
                psn = mpool.tile([128, FT], _dt, tag="ps")  # -Ps
                cs = init_s if j == 0 else prev_ps[:, FT - 1:FT]
                ema(psn[:], xt[:], rds[:], cs, as_)
                prev_pf, prev_ps = pfn, psn

                # r2 = 1 + 1/S straight from the two EMA outputs (critical
                # chain); s3n on GPSIMD in parallel (only 'out' needs it)
                r2 = mpool.tile([128, FT], _dt, tag="q")
                nc.vector._custom_dve(RECIP2N_OP, out=r2[:], in0=pfn[:],
                                      in1=psn[:], s0=float(E), s1=_CHEB0,
                                      imm2=_CHEB1)

                s3n = mpool.tile([128, FT], _dt, tag="s3")  # -(Pf+Ps)
                _tt_split(nc, SPLIT_S, s3n, pfn, psn, _A.add)

                xr = mpool.tile([128, FT], _dt, tag="xr")
                nc.scalar.activation(xr[:], xt[:], _Act.Relu, scale=mh)

                t1 = mpool.tile([128, FT], _dt, tag="t1")
                _tt_split(nc, SPLIT_T1, t1, r2, xr, _A.mult)

                o = xr  # xr dead after t1; out = t1 + (-(Pf+Ps)) (drops -E)
                _tt_split(nc, SPLIT_OUT, o, t1, s3n, _A.add)

                nc.scalar.dma_start(y_d.ap()[:, sl], o[0:64, :])
                nc.scalar.dma_start(y_d.ap()[:, sh], o[64:128, :])

        if reps > 0:
            for _ in range(reps):
                body()
        else:
            body()

    nc.compile()
    return nc


_CACHE: dict = {}


def _get_nc(consts: dict, reps: int = 0):
    key = (tuple(sorted(consts.items())), reps, BUFS, SPLIT_S, SPLIT_T1,
           SPLIT_OUT, FT, W_SLOW)
    if key not in _CACHE:
        _CACHE[key] = _build(consts, reps)
    return _CACHE[key]


def _consts_from_inputs(mu_fast, mu_slow, mix_weight_adapt, mix_weight_hp) -> dict:
    mu_f = _f32(mu_fast)
    mu_s = _f32(mu_slow)
    one = np.float32(1.0)
    ma = np.float32(one / (one + np.exp(np.float32(-mix_weight_adapt))))
    mh = np.float32(one / (one + np.exp(np.float32(-mix_weight_hp))))
    af = one - mu_f
    as_ = one - mu_s
    d = dict(
        af=float(af),
        Q=float(np.float32(mh * ma * mu_f)),
        R=float(np.float32(mh * (one - ma) * mu_s)),
        E=float(np.float32(mh * EPS)),
        mh=float(mh),
        mh_ma=float(np.float32(mh * ma)),
        mh_1ma=float(np.float32(mh * (one - ma))),
    )
    d["as"] = float(as_)
    return d


def _ramps_from_consts(consts: dict) -> np.ndarray:
    """[128, 2*FT] negated decay ramps: col k -> -Q*af^-k | -R*as^-k."""
    k = np.arange(FT, dtype=np.float64)
    rdf = (-consts["Q"] * np.float64(consts["af"]) ** -k).astype(np.float32)
    rds = (-consts["R"] * np.float64(consts["as"]) ** -k).astype(np.float32)
    return np.ascontiguousarray(
        np.broadcast_to(np.concatenate([rdf, rds]), (128, 2 * FT)))


def _make_in_maps(consts: dict, x: np.ndarray) -> list:
    rmp = _ramps_from_consts(consts)
    return [{"x": np.ascontiguousarray(x[b]), "rmp": rmp} for b in range(B)]


def kernel(x, mu_fast, mu_slow, mix_weight_adapt, mix_weight_hp):
    x = np.asarray(x, dtype=np.float32)
    assert x.shape == (B, C, T), x.shape
    consts = _consts_from_inputs(mu_fast, mu_slow, mix_weight_adapt, mix_weight_hp)
    nc = _get_nc(consts)
    res = run_bass_kernel_spmd(nc, _make_in_maps(consts, x),
                               core_ids=list(range(B)))
    return np.stack([res.results[b]["y"] for b in range(B)], axis=0)


if __name__ == "__main__":
    rng = np.random.default_rng(0)
    import math
    FS = 16000.0
    x = rng.standard_normal((B, C, T), dtype=np.float32)
    out = kernel(
        x,
        np.float32(1.0 - math.exp(-1.0 / (FS * 2.0 / 1000.0))),
        np.float32(1.0 - math.exp(-1.0 / (FS * 60.0 / 1000.0))),
        np.float32(0.5),
        np.float32(0.3),
    )
    print(out.shape, out.dtype, np.isfinite(out).all())
